# revision 34
# baseline (speedup 1.0000x reference)
"""Bass/Trainium2 kernel for nn_Block_57140244906730 (MLA attention + noisy top-2
router + 8 capacity-limited SwiGLU experts), distributed over 8 NeuronCores.

Sharding: token-parallel attention (512 tokens/core, kv computed per batch
block), expert-parallel MoE (1 expert/core) with AllGather dispatch/combine.
Capacity cumsum on device via tensor_tensor_scan + triangular matmuls;
dispatch/combine row movement via indirect DMA.
"""
import os
import sys

for _p in ('/root/.axon_site/_ro/trn_rl_repo', '/opt/trn_rl_repo'):
    if os.path.isdir(_p) and _p not in sys.path:
        sys.path.insert(0, _p)

import numpy as np
import ml_dtypes

import concourse.bacc as bacc
import concourse.bass as bass
import concourse.mybir as mybir
import concourse.tile as tile
from concourse.bass_utils import run_bass_kernel_spmd

f32 = mybir.dt.float32
bf16 = mybir.dt.bfloat16
i16 = mybir.dt.int16
i32 = mybir.dt.int32
AF = mybir.ActivationFunctionType
OP = mybir.AluOpType
BF = ml_dtypes.bfloat16

B, T, D = 4, 1024, 1024
NH, HD, LAT = 16, 64, 32
NE, TOPK = 8, 2
HID = 2730
HIDP = 2816           # padded hidden (22 * 128)
HC = HIDP // 128      # 22
N = B * T             # 4096
NCORE = 8
TOK = N // NCORE      # 512
TB = 1024             # batch-block (kv scope)
CAP = N * TOPK // NE  # 1024
DC = D // 128         # 8
P = 128
BIGNEG = 1e30

_CACHE = {}


def _rope(nc, pool, out_bf, in_f32, cos2_ap, sin2_ap):
    """RoPE with rows pre-permuted to [even(32); odd(32)].

    cos2 = [cos; cos], sin2 = [sin; -sin] (both [64, w]).
    tc = in*cos2; tsw = swap-halves(in*sin2)  (DMA partition swap);
    out = tc + tsw:
      out[0:32]  = e*cos + (o * -sin) = e*cos - o*sin
      out[32:64] = o*cos + (e *  sin) = o*cos + e*sin
    """
    tt = nc.vector.tensor_tensor
    w = in_f32.shape[-1]
    tc_ = pool.tile([HD, TB], f32, tag="rope_c")
    ts_ = pool.tile([HD, TB], f32, tag="rope_s")
    tw_ = pool.tile([HD, TB], f32, tag="rope_w")
    tt(tc_[:, :w], in_f32, cos2_ap, op=OP.mult)
    tt(ts_[:, :w], in_f32, sin2_ap, op=OP.mult)
    nc.sync.dma_start(tw_[0:32, :w], ts_[32:64, :w])
    nc.sync.dma_start(tw_[32:64, :w], ts_[0:32, :w])
    tt(out_bf[:, :w], tc_[:, :w], tw_[:, :w], op=OP.add)


def _build():
    nc = bacc.Bacc(None, target_bir_lowering=False, num_devices=NCORE)
    pr = lambda name, shape, dt: nc.declare_dram_parameter(name, list(shape), dt, isOutput=False)

    xq = pr("xq", [P, 4 * D], f32)
    xtb = pr("xtb", [P, DC * TB], f32)
    coskv = pr("coskv", [HD, TB], f32)   # [cos; cos]
    sinkv = pr("sinkv", [HD, TB], f32)   # [sin; -sin]
    epst = pr("epst", [P, 4 * NE], f32)
    eoh = pr("eoh", [P, NE], f32)
    own_sel = pr("own_sel", [P, 1], f32)
    qw = pr("qw", [P, NH * DC * HD], bf16)
    qbt = pr("qbt", [HD, NH], f32)
    kaw = pr("kaw", [P, NH * DC * LAT], bf16)
    kbw = pr("kbw", [LAT, NH * HD], bf16)
    vaw = pr("vaw", [P, NH * DC * LAT], bf16)
    vbw = pr("vbw", [LAT, NH * HD], bf16)
    pjw = pr("pjw", [P, DC * (NH // 2) * P], bf16)   # [p,(m,hp,c)]
    pjb = pr("pjb", [P, DC], f32)
    ln1c = pr("ln1c", [P, DC], f32)
    ln2m = pr("ln2m", [P, D], f32)
    rww = pr("rww", [P, DC * NE], f32)
    rwb = pr("rwb", [1, NE], f32)
    nww = pr("nww", [P, DC * NE], f32)
    nwb = pr("nwb", [1, NE], f32)
    w1t = pr("w1t", [HC, P, 2 * DC * P], bf16)
    w2t = pr("w2t", [P, HC * DC * P], bf16)

    out_x = nc.declare_dram_parameter("out_x", [TOK, D], f32, isOutput=True)
    out_aux = nc.declare_dram_parameter("out_aux", [1, 1], f32, isOutput=True)

    il = nc.inline_tensor
    su128 = il(np.triu(np.ones((P, P), np.float32), 1), name="su128")   # p'<p
    ui128 = il(np.triu(np.ones((P, P), np.float32), 0), name="ui128")   # p'<=p
    id128f = il(np.eye(P, dtype=np.float32), name="id128f")
    id128b = il(np.eye(P, dtype=BF), name="id128b")
    onc = il(np.ones((P, 1), np.float32), name="onc")
    onr = il(np.ones((1, P), np.float32), name="onr")
    onr64 = il(np.ones((1, 64), np.float32), name="onr64")
    er8c = il(np.tile(np.arange(NE, dtype=np.float32), (P, 1)), name="er8c")
    erm9c = il(np.tile(np.arange(NE, dtype=np.float32) - 9.0, (P, 1)), name="erm9c")
    ecm1c = il(np.tile(np.arange(NE, dtype=np.float32) * CAP - 1.0, (P, 1)), name="ecm1c")
    rep16 = il((np.arange(P)[None, :] % 16 == np.arange(16)[:, None]).astype(np.float32),
               name="rep16")

    agx_in = nc.dram_tensor("agx_in", [TOK, D], bf16)
    agx_out = nc.dram_tensor("agx_out", [N, D], bf16, addr_space="Shared")
    agp_in = nc.dram_tensor("agp_in", [TOK, NE], f32)
    agp_out = nc.dram_tensor("agp_out", [N, NE], f32, addr_space="Shared")
    ago_in = nc.dram_tensor("ago_in", [CAP, D], bf16)
    ago_out = nc.dram_tensor("ago_out", [NE * CAP, D], bf16, addr_space="Shared")
    # 32 independent slot->token scatter targets (one per scan column) so the
    # indirect scatters run in parallel; summed on-chip (disjoint writes + zeros)
    tokbufs = [nc.dram_tensor(f"tokb{j}", [CAP + 16], i16) for j in range(32)]

    RG = [list(range(NCORE))]
    mm = nc.tensor.matmul
    tt = nc.vector.tensor_tensor
    ts = nc.vector.tensor_scalar
    stt = nc.vector.scalar_tensor_tensor
    cp = nc.vector.tensor_copy
    act = nc.scalar.activation
    red = nc.vector.tensor_reduce
    X = mybir.AxisListType.X

    with tile.TileContext(nc) as tc:
        import contextlib
        with contextlib.ExitStack() as es:
            kc = es.enter_context(tc.tile_pool(name="consts", bufs=1))
            wt = es.enter_context(tc.tile_pool(name="wts", bufs=1))
            bigp = es.enter_context(tc.tile_pool(name="big", bufs=1))

            def load(pool, name, shape, dt, view):
                t = pool.tile(shape, dt, tag=name)
                nc.sync.dma_start(t[:], view)
                return t

            su_sb = load(kc, "su", [P, P], f32, su128[:])
            ui_sb = load(kc, "ui", [P, P], f32, ui128[:])
            idf_sb = load(kc, "idf", [P, P], f32, id128f[:])
            idb_sb = load(kc, "idb", [P, P], bf16, id128b[:])
            oc_sb = load(kc, "oc", [P, 1], f32, onc[:])
            or_sb = load(kc, "or", [1, P], f32, onr[:])
            # ones row placed at partition 64 (matmul lhsT/rhs base must match;
            # rhs is the softmax-denominator row living at partition 64)
            or64_sb = kc.tile([HD + 1, 64], f32, tag="or64")
            nc.sync.dma_start(or64_sb[HD:HD + 1, :], onr64[:])
            er8_sb = load(kc, "er8", [P, NE], f32, er8c[:])
            erm9_sb = load(kc, "erm9", [P, NE], f32, erm9c[:])
            ecm1_sb = load(kc, "ecm1", [P, NE], f32, ecm1c[:])
            eoh_sb = load(kc, "eoh", [P, NE], f32, eoh[:])
            osel_sb = load(kc, "osel", [P, 1], f32, own_sel[:])
            cos_sb = load(kc, "cos", [HD, TB], f32, coskv[:])
            sin_sb = load(kc, "sin", [HD, TB], f32, sinkv[:])
            ln1_sb = load(kc, "ln1", [P, DC], f32, ln1c[:])
            qbt_sb = load(kc, "qbt", [HD, NH], f32, qbt[:])
            pjb_sb = load(kc, "pjb", [P, DC], f32, pjb[:])
            rwb_sb = load(kc, "rwb", [1, NE], f32, rwb[:])
            nwb_sb = load(kc, "nwb", [1, NE], f32, nwb[:])
            eps_sb = load(kc, "eps", [P, 4, NE], f32, epst[:].rearrange("p (j e) -> p j e", j=4))
            kb_sb = load(kc, "kb", [LAT, NH, HD], bf16, kbw[:].rearrange("l (h e) -> l h e", h=NH))
            vb_sb = load(kc, "vb", [LAT, NH, HD], bf16, vbw[:].rearrange("l (h e) -> l h e", h=NH))

            # zero the 32 scatter buffers up front (no deps -> overlaps attention)
            ztok0 = kc.tile([1, CAP + 16], i16, tag="ztok0")
            nc.vector.memset(ztok0[:], 0)
            for _j in range(32):
                nc.sync.dma_start(tokbufs[_j][None, :], ztok0[:])

            ln2_sb = load(wt, "ln2", [P, D], f32, ln2m[:])
            rw_sb = load(wt, "rw", [P, DC, NE], f32, rww[:].rearrange("p (d e) -> p d e", d=DC))
            nw_sb = load(wt, "nw", [P, DC, NE], f32, nww[:].rearrange("p (d e) -> p d e", d=DC))

            x2r = bigp.tile([P, 4, D], f32, tag="x2r")
            probs = bigp.tile([P, 4, NE], f32, tag="probs")
            selt = bigp.tile([P, 4, NE], f32, tag="selt")
            idx_lo32 = bigp.tile([P, 4], i32, tag="idxlo")
            idx_hi32 = bigp.tile([P, 4], i32, tag="idxhi")
            gate_lo = bigp.tile([P, 4], f32, tag="glo")
            gate_hi = bigp.tile([P, 4], f32, tag="ghi")

            qw_v = qw[:].rearrange("p (h d e) -> p h d e", h=NH, d=DC)
            ka_v = kaw[:].rearrange("p (h d e) -> p h d e", h=NH, d=DC)
            va_v = vaw[:].rearrange("p (h d e) -> p h d e", h=NH, d=DC)
            pj_v = pjw[:].rearrange("p (m h c) -> p m h c", m=DC, h=NH // 2)
            xtb_v = xtb[:].rearrange("p (d t) -> p d t", d=DC)

            # ================= attention =================
            with tc.tile_pool(name="attp", bufs=1) as aw, \
                 tc.tile_pool(name="at1", bufs=1) as at1, \
                 tc.tile_pool(name="at2", bufs=2) as at2:
                xnT = aw.tile([P, DC, TB], bf16, tag="xnT")
                oTn = aw.tile([P, NH // 2, 512], bf16, tag="oTn")

                # ---- rms1 (xtb streamed) ----
                with tc.tile_pool(name="ps_r1", bufs=1, space="PSUM") as psr1:
                    ssq_ps = psr1.tile([1, TB], f32, tag="ssq")
                    for dc in range(DC):
                        xt = at2.tile([P, TB], f32, tag="xt")
                        nc.sync.dma_start(xt[:], xtb_v[:, dc, :])
                        sq = at2.tile([P, TB], f32, tag="sq")
                        tt(sq[:], xt[:], xt[:], op=OP.mult)
                        for hf in range(2):
                            mm(ssq_ps[:, 512 * hf:512 * (hf + 1)], lhsT=oc_sb[:],
                               rhs=sq[:, 512 * hf:512 * (hf + 1)],
                               start=(dc == 0), stop=(dc == DC - 1))
                    sq0 = at1.tile([1, TB], f32, tag="sq0")
                    ts(sq0[:], ssq_ps[:], 1.0 / D, 1e-5, op0=OP.mult, op1=OP.add)
                    act(sq0[:], sq0[:], AF.Sqrt)
                    rstd1 = at1.tile([1, TB], f32, tag="rstd1")
                    nc.vector.reciprocal(rstd1[:], sq0[:])
                    rb_ps = psr1.tile([P, TB], f32, tag="rb")
                    for hf in range(2):
                        mm(rb_ps[:, 512 * hf:512 * (hf + 1)], lhsT=or_sb[:],
                           rhs=rstd1[:, 512 * hf:512 * (hf + 1)], start=True, stop=True)
                    rstdb = at1.tile([P, TB], f32, tag="rstdb")
                    cp(rstdb[:], rb_ps[:])
                    for dc in range(DC):
                        xt = at2.tile([P, TB], f32, tag="xt")
                        nc.sync.dma_start(xt[:], xtb_v[:, dc, :])
                        stt(xnT[:, dc, :], xt[:], ln1_sb[:, dc:dc + 1], rstdb[:],
                            op0=OP.mult, op1=OP.mult)

                # ---- heads ----
                with tc.tile_pool(name="ps_h", bufs=1, space="PSUM") as psh, \
                     tc.tile_pool(name="ps_h2", bufs=2, space="PSUM") as psh2:
                    for h in range(NH):
                        qw_h = at2.tile([P, DC, HD], bf16, tag="qw_h")
                        nc.sync.dma_start(qw_h[:], qw_v[:, h, :, :])
                        ka_h = at2.tile([P, DC, LAT], bf16, tag="ka_h")
                        nc.sync.dma_start(ka_h[:], ka_v[:, h, :, :])
                        va_h = at2.tile([P, DC, LAT], bf16, tag="va_h")
                        nc.sync.dma_start(va_h[:], va_v[:, h, :, :])

                        q_ps = psh.tile([HD, 512], f32, tag="q_ps")
                        for dc in range(DC):
                            mm(q_ps[:], lhsT=qw_h[:, dc, :], rhs=xnT[:, dc, 0:512],
                               start=(dc == 0), stop=(dc == DC - 1))
                        q_sb = at1.tile([HD, TB], f32, tag="qk_pre")
                        act(q_sb[:, 0:512], q_ps[:], AF.Identity, bias=qbt_sb[:, h:h + 1])
                        qr = at2.tile([HD, 512], bf16, tag="qr")
                        _rope(nc, at1, qr, q_sb[:, 0:512], cos_sb[:, 0:512], sin_sb[:, 0:512])

                        latk = at2.tile([LAT, TB], bf16, tag="latk")
                        for hf in range(2):
                            lat_ps = psh.tile([LAT, 512], f32, tag="lat_ps")
                            for dc in range(DC):
                                mm(lat_ps[:], lhsT=ka_h[:, dc, :],
                                   rhs=xnT[:, dc, 512 * hf:512 * (hf + 1)],
                                   start=(dc == 0), stop=(dc == DC - 1))
                            cp(latk[:, 512 * hf:512 * (hf + 1)], lat_ps[:])
                        k_sb = at1.tile([HD, TB], f32, tag="k_pre")
                        for hf in range(2):
                            kp_ps = psh.tile([HD, 512], f32, tag="kp_ps")
                            mm(kp_ps[:], lhsT=kb_sb[:, h, :],
                               rhs=latk[:, 512 * hf:512 * (hf + 1)], start=True, stop=True)
                            cp(k_sb[:, 512 * hf:512 * (hf + 1)], kp_ps[:])
                        kr = at2.tile([HD, TB], bf16, tag="kr")
                        _rope(nc, at1, kr, k_sb[:], cos_sb[:], sin_sb[:])

                        latv = at2.tile([LAT, TB], bf16, tag="latv")
                        for hf in range(2):
                            lat_ps = psh.tile([LAT, 512], f32, tag="lat_ps")
                            for dc in range(DC):
                                mm(lat_ps[:], lhsT=va_h[:, dc, :],
                                   rhs=xnT[:, dc, 512 * hf:512 * (hf + 1)],
                                   start=(dc == 0), stop=(dc == DC - 1))
                            cp(latv[:, 512 * hf:512 * (hf + 1)], lat_ps[:])
                        # vaug cols 0:64 = v, col 64 = ones (denominator row at
                        # partition 64 of oT_ps)
                        vaug = at2.tile([P, 8, HD + 1], bf16, tag="vaug")
                        for kcc in range(8):
                            v_ps = psh.tile([P, 512], f32, tag="aux_ps")
                            mm(v_ps[:, 0:HD], lhsT=latv[:, 128 * kcc:128 * (kcc + 1)],
                               rhs=vb_sb[:, h, :], start=True, stop=True)
                            cp(vaug[:, kcc, 0:HD], v_ps[:, 0:HD])
                        nc.vector.memset(vaug[:, :, HD:HD + 1], 1.0)

                        p_sb = at2.tile([P, 8, 512], bf16, tag="p_sb")
                        for kcc in range(8):
                            s_ps = psh2.tile([P, 512], f32, tag="s_ps")
                            mm(s_ps[:], lhsT=kr[:, 128 * kcc:128 * (kcc + 1)], rhs=qr[:],
                               start=True, stop=True)
                            act(p_sb[:, kcc, :], s_ps[:], AF.Exp, scale=0.125)
                        oT_ps = psh.tile([HD + 1, 512], f32, tag="oT_ps")
                        for kcc in range(8):
                            mm(oT_ps[:], lhsT=vaug[:, kcc, :], rhs=p_sb[:, kcc, :],
                               start=(kcc == 0), stop=(kcc == 7))
                        rs64 = at1.tile([HD + 1, 512], f32, tag="rs64")
                        nc.vector.reciprocal(rs64[HD:HD + 1, :], oT_ps[HD:HD + 1, :])
                        rb2_ps = psh.tile([P, 512], f32, tag="aux_ps")
                        mm(rb2_ps[0:HD, :], lhsT=or64_sb[HD:HD + 1, :], rhs=rs64[HD:HD + 1, :],
                           start=True, stop=True)
                        rb2 = at1.tile([HD, 512], f32, tag="rb2s")
                        cp(rb2[:], rb2_ps[0:HD, :])
                        oTn_h = at2.tile([HD, 512], bf16, tag="oTn_h")
                        tt(oTn_h[:], oT_ps[0:HD, :], rb2[:], op=OP.mult)
                        r0 = (h % 2) * HD
                        nc.sync.dma_start(oTn[r0:r0 + HD, h // 2, :], oTn_h[:])

                # ---- proj + residual (rows) ----
                with tc.tile_pool(name="ps_pj", bufs=1, space="PSUM") as pspj, \
                     tc.tile_pool(name="ps_tr", bufs=2, space="PSUM") as pstr:
                    # proj computed per m (all tokens), transposed per t4
                    pjT_all = aw.tile([P, DC, 512], bf16, tag="pjT_all")
                    for m in range(DC):
                        pj_sb_m = at2.tile([P, NH // 2, P], bf16, tag="pj_w")
                        nc.sync.dma_start(pj_sb_m[:], pj_v[:, m, :, :])
                        pj_ps = pspj.tile([P, 512], f32, tag="pj_ps")
                        for hp in range(NH // 2):
                            mm(pj_ps[:], lhsT=pj_sb_m[:, hp, :], rhs=oTn[:, hp, :],
                               start=(hp == 0), stop=(hp == NH // 2 - 1))
                        act(pjT_all[:, m, :], pj_ps[:], AF.Identity, bias=pjb_sb[:, m:m + 1])
                    for t4 in range(4):
                        xqt = at2.tile([P, D], f32, tag="xqt")
                        nc.sync.dma_start(xqt[:], xq[:].rearrange("p (j d) -> p j d", j=4)[:, t4, :])
                        for m in range(DC):
                            tr_ps = pstr.tile([P, P], bf16, tag="tr")
                            nc.tensor.transpose(tr_ps[:], pjT_all[:, m, 128 * t4:128 * (t4 + 1)], idb_sb[:])
                            tt(x2r[:, t4, 128 * m:128 * (m + 1)], tr_ps[:],
                               xqt[:, 128 * m:128 * (m + 1)], op=OP.add)

            # ================= rms2 / xn2 / AG(x) / router =================
            with tc.tile_pool(name="n2", bufs=2) as n2p, \
                 tc.tile_pool(name="n2s", bufs=1) as n2s, \
                 tc.tile_pool(name="xn2Tp", bufs=1) as xtp, \
                 tc.tile_pool(name="ps_n2", bufs=2, space="PSUM") as psn:
                xn2T = xtp.tile([P, DC, 512], f32, tag="xn2T")
                for j2 in range(4):
                    sq = n2p.tile([P, D], f32, tag="sq2")
                    tt(sq[:], x2r[:, j2, :], x2r[:, j2, :], op=OP.mult)
                    ssq = n2p.tile([P, 1], f32, tag="ssq2")
                    red(ssq[:], sq[:], axis=X, op=OP.add)
                    rstd2 = n2p.tile([P, 1], f32, tag="rstd2")
                    ts(rstd2[:], ssq[:], 1.0 / D, 1e-5, op0=OP.mult, op1=OP.add)
                    act(rstd2[:], rstd2[:], AF.Sqrt)
                    nc.vector.reciprocal(rstd2[:], rstd2[:])
                    xn2 = n2p.tile([P, D], f32, tag="xn2")
                    stt(xn2[:], x2r[:, j2, :], rstd2[:], ln2_sb[:], op0=OP.mult, op1=OP.mult)
                    xn2b = n2p.tile([P, D], bf16, tag="xn2b")
                    cp(xn2b[:], xn2[:])
                    nc.sync.dma_start(agx_in[128 * j2:128 * (j2 + 1), :], xn2b[:])
                    for m in range(DC):
                        tr_ps = psn.tile([P, P], f32, tag="trn")
                        nc.tensor.transpose(tr_ps[:], xn2[:, 128 * m:128 * (m + 1)], idf_sb[:])
                        cp(xn2T[:, m, 128 * j2:128 * (j2 + 1)], tr_ps[:])
                nc.gpsimd.collective_compute(
                    "AllGather", OP.bypass, replica_groups=RG,
                    ins=[agx_in[:]], outs=[agx_out[:]])

                for j2 in range(4):
                    lg_ps = psn.tile([P, NE], f32, tag="lg")
                    npp_ps = psn.tile([P, NE], f32, tag="npp")
                    for dc in range(DC):
                        mm(lg_ps[:], lhsT=xn2T[:, dc, 128 * j2:128 * (j2 + 1)],
                           rhs=rw_sb[:, dc, :], start=(dc == 0), stop=False)
                        mm(npp_ps[:], lhsT=xn2T[:, dc, 128 * j2:128 * (j2 + 1)],
                           rhs=nw_sb[:, dc, :], start=(dc == 0), stop=False)
                    mm(lg_ps[:], lhsT=or_sb[:], rhs=rwb_sb[:], start=False, stop=True)
                    mm(npp_ps[:], lhsT=or_sb[:], rhs=nwb_sb[:], start=False, stop=True)
                    # softplus via Newton on e^y = 1 + e^x (no Softplus table on
                    # this toolchain): y_{n+1} = y_n - 1 + (1+e^x) e^{-y_n}
                    a_t = n2p.tile([P, NE], f32, tag="a_t")
                    act(a_t[:], npp_ps[:], AF.Exp)
                    ts(a_t[:], a_t[:], 1.0, None, op0=OP.add)
                    sp = n2p.tile([P, NE], f32, tag="sp")
                    act(sp[:], npp_ps[:], AF.Relu)
                    for _it in range(5):
                        t_ = n2p.tile([P, NE], f32, tag="t_nwt")
                        act(t_[:], sp[:], AF.Exp, scale=-1.0)
                        tt(t_[:], t_[:], a_t[:], op=OP.mult)
                        stt(sp[:], t_[:], -1.0, sp[:], op0=OP.add, op1=OP.add)
                    noisy = n2p.tile([P, NE], f32, tag="noisy")
                    tt(noisy[:], eps_sb[:, j2, :], sp[:], op=OP.mult)
                    tt(noisy[:], noisy[:], lg_ps[:], op=OP.add)
                    m1 = n2p.tile([P, 1], f32, tag="m1")
                    red(m1[:], noisy[:], axis=X, op=OP.max)
                    eq = n2p.tile([P, NE], f32, tag="eq")
                    ts(eq[:], noisy[:], m1[:], None, op0=OP.is_equal)
                    msk = n2p.tile([P, NE], f32, tag="msk")
                    stt(msk[:], eq[:], -BIGNEG, noisy[:], op0=OP.mult, op1=OP.add)
                    m2 = n2p.tile([P, 1], f32, tag="m2")
                    red(m2[:], msk[:], axis=X, op=OP.max)
                    ts(selt[:, j2, :], noisy[:], m2[:], None, op0=OP.is_ge)
                    m1n = n2p.tile([P, 1], f32, tag="m1n")
                    ts(m1n[:], m1[:], -1.0, None, op0=OP.mult)
                    pe = n2p.tile([P, NE], f32, tag="pe")
                    act(pe[:], noisy[:], AF.Exp, bias=m1n[:])
                    tt(pe[:], pe[:], selt[:, j2, :], op=OP.mult)
                    z = n2p.tile([P, 1], f32, tag="z")
                    red(z[:], pe[:], axis=X, op=OP.add)
                    rz = n2p.tile([P, 1], f32, tag="rz")
                    nc.vector.reciprocal(rz[:], z[:])
                    ts(probs[:, j2, :], pe[:], rz[:], None, op0=OP.mult)
                    pjo = n2p.tile([P, NE], f32, tag="pjo")
                    cp(pjo[:], probs[:, j2, :])
                    nc.sync.dma_start(agp_in[128 * j2:128 * (j2 + 1), :], pjo[:])
                nc.gpsimd.collective_compute(
                    "AllGather", OP.bypass, replica_groups=RG,
                    ins=[agp_in[:]], outs=[agp_out[:]])

            # ================= global routing / capacity / aux =================
            with tc.tile_pool(name="rt", bufs=1) as rt, \
                 tc.tile_pool(name="rt2", bufs=4) as rt2, \
                 tc.tile_pool(name="ps_rt", bufs=1, space="PSUM") as psr, \
                 tc.tile_pool(name="ps_rt2", bufs=2, space="PSUM") as psr2:
                pg = rt.tile([P, 32, NE], f32, tag="pg")
                nc.sync.dma_start(pg[:], agp_out[:].rearrange("(p j) e -> p j e", p=P))
                peo = rt.tile([P, NE, 32], f32, tag="peo")
                cp(peo[:], pg[:].transpose([0, 2, 1]))
                meo = rt.tile([P, NE, 32], f32, tag="meo")
                ts(meo[:], peo[:], 0.0, None, op0=OP.is_gt)
                tots = rt.tile([P, NE], f32, tag="tots")
                red(tots[:], meo[:], axis=X, op=OP.add)
                gpoff_ps = psr.tile([P, NE], f32, tag="gpoff")
                mm(gpoff_ps[:], lhsT=su_sb[:], rhs=tots[:], start=True, stop=True)
                gpoff = rt.tile([P, NE], f32, tag="gpoffs")
                cp(gpoff[:], gpoff_ps[:])

                tmpe = rt.tile([P, 32, NE], f32, tag="tmpe")
                tt(tmpe[:], pg[:], eoh_sb[:].unsqueeze(1).broadcast_to([P, 32, NE]), op=OP.mult)
                gmine = rt.tile([P, 32], f32, tag="gmine")
                red(gmine[:], tmpe[:], axis=X, op=OP.add)
                m_mine = rt.tile([P, 32], f32, tag="mmine")
                ts(m_mine[:], gmine[:], 0.0, None, op0=OP.is_gt)
                scan = rt.tile([P, 32], f32, tag="scan")
                nc.vector.tensor_tensor_scan(scan[:], m_mine[:], m_mine[:], 0.0,
                                             op0=OP.add, op1=OP.bypass)
                gpm = rt.tile([P, NE], f32, tag="gpm")
                tt(gpm[:], gpoff[:], eoh_sb[:], op=OP.mult)
                gpm1 = rt.tile([P, 1], f32, tag="gpm1")
                red(gpm1[:], gpm[:], axis=X, op=OP.add)
                incl = rt.tile([P, 32], f32, tag="incl")
                ts(incl[:], scan[:], gpm1[:], None, op0=OP.add)
                slot = rt.tile([P, 32], f32, tag="slot")
                ts(slot[:], incl[:], -1.0, None, op0=OP.add)
                ts(slot[:], slot[:], float(CAP), None, op0=OP.min)
                ts(slot[:], slot[:], -float(CAP), None, op0=OP.add)
                tt(slot[:], slot[:], m_mine[:], op=OP.mult)
                ts(slot[:], slot[:], float(CAP), None, op0=OP.add)
                # --- tok list: 32 parallel per-column indirect scatters into
                # 32 zeroed buffers, later summed (disjoint nonzeros) ---
                slot32 = rt.tile([P, 32], i32, tag="slot32")
                cp(slot32[:], slot[:])
                iot = rt.tile([P, 32], i16, tag="iot")
                nc.gpsimd.iota(iot[:], pattern=[[1, 32]], base=0, channel_multiplier=32)
                for j in range(32):
                    nc.gpsimd.indirect_dma_start(
                        out=tokbufs[j][:, None],
                        out_offset=bass.IndirectOffsetOnAxis(ap=slot32[:, j:j + 1], axis=0),
                        in_=iot[:, j:j + 1], in_offset=None)
                # aux loss (identical on every core)
                jsum = rt.tile([P, NE], f32, tag="jsum")
                red(jsum[:], peo[:], axis=X, op=OP.add)
                cs_ps = psr.tile([1, NE], f32, tag="cs")
                mm(cs_ps[:], lhsT=oc_sb[:], rhs=jsum[:], start=True, stop=True)
                auxv = rt.tile([1, NE], f32, tag="auxv")
                ts(auxv[:], cs_ps[:], 1.0 / N, -1.0 / NE, op0=OP.mult, op1=OP.add)
                act(auxv[:], auxv[:], AF.Square)
                aux1 = rt.tile([1, 1], f32, tag="aux1")
                red(aux1[:], auxv[:], axis=X, op=OP.add)
                nc.sync.dma_start(out_aux[:], aux1[:])

                # combine-side indices (own tokens, tile-major)
                mown = rt.tile([P, 4, NE], f32, tag="mown")
                ts(mown[:], probs[:], 0.0, None, op0=OP.is_gt)
                inclo = rt.tile([P, 4, NE], f32, tag="inclo")
                coffacc_ps = psr.tile([1, NE], f32, tag="coffacc")
                mm(coffacc_ps[:], lhsT=osel_sb[:], rhs=gpoff[:], start=True, stop=False)
                for j2 in range(4):
                    coff = rt2.tile([1, NE], f32, tag="coff")
                    cp(coff[:], coffacc_ps[:])
                    mj = rt2.tile([P, NE], f32, tag="mj")
                    cp(mj[:], mown[:, j2, :])
                    csc_ps = psr2.tile([P, NE], f32, tag="csc")
                    mm(csc_ps[:], lhsT=ui_sb[:], rhs=mj[:], start=True, stop=False)
                    mm(csc_ps[:], lhsT=or_sb[:], rhs=coff[:], start=False, stop=True)
                    cp(inclo[:, j2, :], csc_ps[:])
                    if j2 < 3:
                        mm(coffacc_ps[:], lhsT=oc_sb[:], rhs=mj[:], start=False,
                           stop=(j2 == 2))
                keep = rt.tile([P, 4, NE], f32, tag="keep")
                ts(keep[:], inclo[:], float(CAP), None, op0=OP.is_le)
                tt(keep[:], keep[:], mown[:], op=OP.mult)
                gate0 = rt.tile([P, 4, NE], f32, tag="gate0")
                tt(gate0[:], probs[:], keep[:], op=OP.mult)
                idxc = rt.tile([P, 4, NE], f32, tag="idxc")
                ts(idxc[:], inclo[:], float(CAP), None, op0=OP.min)
                tt(idxc[:], idxc[:], ecm1_sb[:].unsqueeze(1).broadcast_to([P, 4, NE]), op=OP.add)
                erb = er8_sb[:].unsqueeze(1).broadcast_to([P, 4, NE])
                u = rt.tile([P, 4, NE], f32, tag="u")
                tt(u[:], erm9_sb[:].unsqueeze(1).broadcast_to([P, 4, NE]), selt[:], op=OP.mult)
                ts(u[:], u[:], 9.0, None, op0=OP.add)
                elo = rt.tile([P, 4], f32, tag="elo")
                red(elo[:], u[:], axis=X, op=OP.min)
                wv = rt.tile([P, 4, NE], f32, tag="wv")
                tt(wv[:], erb, selt[:], op=OP.mult)
                ehi = rt.tile([P, 4], f32, tag="ehi")
                red(ehi[:], wv[:], axis=X, op=OP.max)
                for nm, ev, idxo, gto in (("lo", elo, idx_lo32, gate_lo),
                                          ("hi", ehi, idx_hi32, gate_hi)):
                    oh = rt.tile([P, 4, NE], f32, tag="oh" + nm)
                    tt(oh[:], erb, ev[:].unsqueeze(2).broadcast_to([P, 4, NE]), op=OP.is_equal)
                    tmp = rt.tile([P, 4, NE], f32, tag="tmq" + nm)
                    tt(tmp[:], gate0[:], oh[:], op=OP.mult)
                    red(gto[:], tmp[:], axis=X, op=OP.add)
                    tt(tmp[:], idxc[:], oh[:], op=OP.mult)
                    idxf = rt2.tile([P, 4], f32, tag="idxf")
                    red(idxf[:], tmp[:], axis=X, op=OP.add)
                    cp(idxo[:], idxf[:])

            # ================= expert =================
            with tc.tile_pool(name="ex", bufs=1) as ex, \
                 tc.tile_pool(name="w1p", bufs=2) as w1p, \
                 tc.tile_pool(name="ex2", bufs=2) as ex2, \
                 tc.tile_pool(name="oerp", bufs=2) as oerp, \
                 tc.tile_pool(name="ps_e", bufs=1, space="PSUM") as pse, \
                 tc.tile_pool(name="ps_e2", bufs=2, space="PSUM") as pse2:
                w2_sb = ex.tile([P, HC, DC, P], bf16, tag="w2")
                nc.sync.dma_start(w2_sb[:], w2t[:].rearrange("p (h m c) -> p h m c", h=HC, m=DC))
                # sum the 32 scatter buffers (each zero except its own slots)
                tk0 = ex.tile([P, 8], i16, tag="tk0")
                nc.sync.dma_start(tk0[:], tokbufs[0][0:CAP].rearrange("(p j) -> p j", p=P))
                tok16 = ex.tile([P, 8], i16, tag="tok16")
                tk1 = ex2.tile([P, 8], i16, tag="tk1")
                nc.sync.dma_start(tk1[:], tokbufs[1][0:CAP].rearrange("(p j) -> p j", p=P))
                tt(tok16[:], tk0[:], tk1[:], op=OP.add)
                for j in range(2, 32):
                    tkj = ex2.tile([P, 8], i16, tag="tk1")
                    nc.sync.dma_start(tkj[:], tokbufs[j][0:CAP].rearrange("(p j) -> p j", p=P))
                    tt(tok16[:], tok16[:], tkj[:], op=OP.add)
                tok32 = ex.tile([P, 8], i32, tag="tok32")
                cp(tok32[:], tok16[:])
                xeT = ex.tile([P, DC, CAP], bf16, tag="xeT")
                for j in range(8):
                    xe = ex2.tile([P, D], bf16, tag="xe")
                    nc.gpsimd.indirect_dma_start(
                        out=xe[:], out_offset=None, in_=agx_out[:],
                        in_offset=bass.IndirectOffsetOnAxis(ap=tok32[:, j:j + 1], axis=0))
                    for dc in range(DC):
                        tr_ps = pse2.tile([P, P], bf16, tag="trx")
                        nc.tensor.transpose(tr_ps[:], xe[:, 128 * dc:128 * (dc + 1)], idb_sb[:])
                        cp(xeT[:, dc, 128 * j:128 * (j + 1)], tr_ps[:])
                g_sb = ex.tile([P, HC, 2, 512], bf16, tag="g")
                for hc in range(HC):
                    w1_sb = w1p.tile([P, 2, DC, P], bf16, tag="w1")
                    nc.sync.dma_start(w1_sb[:], w1t[hc].rearrange("p (h d c) -> p h d c", h=2, d=DC))
                    for t5 in range(2):
                        h1_ps = pse.tile([P, 512], f32, tag="h1")
                        h2_ps = pse.tile([P, 512], f32, tag="h2")
                        for dc in range(DC):
                            mm(h1_ps[:], lhsT=w1_sb[:, 0, dc, :],
                               rhs=xeT[:, dc, 512 * t5:512 * (t5 + 1)],
                               start=(dc == 0), stop=(dc == DC - 1))
                            mm(h2_ps[:], lhsT=w1_sb[:, 1, dc, :],
                               rhs=xeT[:, dc, 512 * t5:512 * (t5 + 1)],
                               start=(dc == 0), stop=(dc == DC - 1))
                        sil = ex2.tile([P, 512], f32, tag="sil")
                        act(sil[:], h1_ps[:], AF.Silu)
                        tt(g_sb[:, hc, t5, :], sil[:], h2_ps[:], op=OP.mult)
                ago_v = ago_in[:].rearrange("(p j) d -> p j d", p=P)
                for t5 in range(2):
                    oers = [oerp.tile([P, D], bf16, tag=f"oer{q}", name=f"oer{q}_{t5}")
                            for q in range(4)]
                    for m in range(DC):
                        oe_ps = pse.tile([P, 512], f32, tag="oe")
                        for hc in range(HC):
                            mm(oe_ps[:], lhsT=w2_sb[:, hc, m, :], rhs=g_sb[:, hc, t5, :],
                               start=(hc == 0), stop=(hc == HC - 1))
                        oeT = ex2.tile([P, 512], bf16, tag="oeT")
                        cp(oeT[:], oe_ps[:])
                        for q in range(4):
                            tr_ps = pse2.tile([P, P], bf16, tag="tro")
                            nc.tensor.transpose(tr_ps[:], oeT[:, 128 * q:128 * (q + 1)], idb_sb[:])
                            cp(oers[q][:, 128 * m:128 * (m + 1)], tr_ps[:])
                    # slot s = 8*p + j' with j' = 4*t5 + q  (tok loaded as [p, j])
                    for q in range(4):
                        nc.sync.dma_start(ago_v[:, 4 * t5 + q, :], oers[q][:])
                nc.gpsimd.collective_compute(
                    "AllGather", OP.bypass, replica_groups=RG,
                    ins=[ago_in[:]], outs=[ago_out[:]])

            # ================= combine =================
            with tc.tile_pool(name="cb", bufs=2) as cb:
                for j2 in range(4):
                    glo = cb.tile([P, D], bf16, tag="glo_t")
                    nc.gpsimd.indirect_dma_start(
                        out=glo[:], out_offset=None, in_=ago_out[:],
                        in_offset=bass.IndirectOffsetOnAxis(ap=idx_lo32[:, j2:j2 + 1], axis=0))
                    ghi = cb.tile([P, D], bf16, tag="ghi_t")
                    nc.gpsimd.indirect_dma_start(
                        out=ghi[:], out_offset=None, in_=ago_out[:],
                        in_offset=bass.IndirectOffsetOnAxis(ap=idx_hi32[:, j2:j2 + 1], axis=0))
                    acc = cb.tile([P, D], f32, tag="acc")
                    stt(acc[:], glo[:], gate_lo[:, j2:j2 + 1], x2r[:, j2, :],
                        op0=OP.mult, op1=OP.add)
                    out_t = cb.tile([P, D], f32, tag="out_t")
                    stt(out_t[:], ghi[:], gate_hi[:, j2:j2 + 1], acc[:],
                        op0=OP.mult, op1=OP.add)
                    nc.sync.dma_start(out_x[128 * j2:128 * (j2 + 1), :], out_t[:])

    nc.compile()
    return nc


# ======================= host side =======================

def _rope_tables():
    inv = 1.0 / (10000.0 ** (np.arange(0, HD, 2, dtype=np.float32) / HD))
    ang = np.arange(T, dtype=np.float32)[:, None] * inv[None, :]
    return np.cos(ang).astype(np.float32), np.sin(ang).astype(np.float32)


def _perm_cols(a):
    idx = np.concatenate([np.arange(0, HD, 2), np.arange(1, HD, 2)])
    return a[..., idx]


def _prep_shared(inp):
    qwp = _perm_cols(np.asarray(inp['q_w'], np.float32))
    qbp = _perm_cols(np.asarray(inp['q_b'], np.float32))
    kbp = _perm_cols(np.asarray(inp['kb_w'], np.float32))
    cos, sin = _rope_tables()

    pk = {}
    pk['qw'] = np.ascontiguousarray(
        qwp.reshape(NH, DC, P, HD).transpose(2, 0, 1, 3).reshape(P, NH * DC * HD)).astype(BF)
    pk['qbt'] = np.ascontiguousarray(qbp.T).astype(np.float32)
    pk['kaw'] = np.ascontiguousarray(
        np.asarray(inp['ka_w'], np.float32).reshape(NH, DC, P, LAT).transpose(2, 0, 1, 3).reshape(P, NH * DC * LAT)).astype(BF)
    pk['vaw'] = np.ascontiguousarray(
        np.asarray(inp['va_w'], np.float32).reshape(NH, DC, P, LAT).transpose(2, 0, 1, 3).reshape(P, NH * DC * LAT)).astype(BF)
    pk['kbw'] = np.ascontiguousarray(kbp.transpose(1, 0, 2).reshape(LAT, NH * HD)).astype(BF)
    pk['vbw'] = np.ascontiguousarray(
        np.asarray(inp['vb_w'], np.float32).transpose(1, 0, 2).reshape(LAT, NH * HD)).astype(BF)
    # pjw[p, m, hp, c] = proj_w[128*hp + p, 128*m + c]
    pk['pjw'] = np.ascontiguousarray(
        np.asarray(inp['proj_w'], np.float32).reshape(NH // 2, P, DC, P)
        .transpose(1, 2, 0, 3).reshape(P, DC * (NH // 2) * P)).astype(BF)
    pk['pjb'] = np.ascontiguousarray(np.asarray(inp['proj_b'], np.float32).reshape(DC, P).T)
    pk['ln1c'] = np.ascontiguousarray(np.asarray(inp['ln1_w'], np.float32).reshape(DC, P).T)
    pk['ln2m'] = np.tile(np.asarray(inp['ln2_w'], np.float32)[None, :], (P, 1))
    pk['rww'] = np.ascontiguousarray(
        np.asarray(inp['route_w'], np.float32).reshape(DC, P, NE).transpose(1, 0, 2).reshape(P, DC * NE))
    pk['nww'] = np.ascontiguousarray(
        np.asarray(inp['noise_w'], np.float32).reshape(DC, P, NE).transpose(1, 0, 2).reshape(P, DC * NE))
    pk['rwb'] = np.asarray(inp['route_b'], np.float32).reshape(1, NE)
    pk['nwb'] = np.asarray(inp['noise_b'], np.float32).reshape(1, NE)
    pk['cos'] = cos
    pk['sin'] = sin
    return pk


def _prep_expert(inp, e):
    w1 = np.asarray(inp['swiglu_w'][e], np.float32)
    w2 = np.asarray(inp['down_w'][e], np.float32)
    h1 = np.zeros((D, HIDP), np.float32); h1[:, :HID] = w1[:, :HID]
    h2 = np.zeros((D, HIDP), np.float32); h2[:, :HID] = w1[:, HID:]
    w2p = np.zeros((HIDP, D), np.float32); w2p[:HID] = w2
    h1r = h1.reshape(DC, P, HC, P)
    h2r = h2.reshape(DC, P, HC, P)
    w1tt = np.stack([h1r, h2r], axis=0)          # [half, dc, p, hc, c]
    w1tt = w1tt.transpose(3, 2, 0, 1, 4)         # [hc, p, half, dc, c]
    w1tt = np.ascontiguousarray(w1tt.reshape(HC, P, 2 * DC * P)).astype(BF)
    w2r = w2p.reshape(HC, P, DC, P).transpose(1, 0, 2, 3)
    w2tt = np.ascontiguousarray(w2r.reshape(P, HC * DC * P)).astype(BF)
    return w1tt, w2tt


def _prep_inputs(inp):
    x = np.asarray(inp['x'], np.float32)
    eps = np.asarray(inp['noise_eps'], np.float32).reshape(N, NE)
    pk = _prep_shared(inp)
    cos, sin = pk['cos'], pk['sin']
    in_maps = []
    for c in range(NCORE):
        b, half = c // 2, c % 2
        s_own = slice(512 * half, 512 * (half + 1))
        s_oth = slice(512 * (1 - half), 512 * (2 - half))
        xb = x[b]
        xq_own = xb[s_own]
        perm_cos = np.concatenate([cos[s_own], cos[s_oth]], 0)
        perm_sin = np.concatenate([sin[s_own], sin[s_oth]], 0)
        xtb_own = np.concatenate([xb[s_own], xb[s_oth]], 0)
        w1tt, w2tt = _prep_expert(inp, c)
        eoh_m = np.zeros((P, NE), np.float32); eoh_m[:, c] = 1.0
        osel = np.zeros((P, 1), np.float32); osel[16 * c, 0] = 1.0
        m = {
            'xq': np.ascontiguousarray(xq_own.reshape(4, P, D).transpose(1, 0, 2).reshape(P, 4 * D)),
            'xtb': np.ascontiguousarray(xtb_own.T.reshape(DC, P, TB).transpose(1, 0, 2).reshape(P, DC * TB)),
            'coskv': np.ascontiguousarray(np.concatenate([perm_cos.T, perm_cos.T], 0)),
            'sinkv': np.ascontiguousarray(np.concatenate([perm_sin.T, -perm_sin.T], 0)),
            'epst': np.ascontiguousarray(
                eps[512 * c:512 * (c + 1)].reshape(4, P, NE).transpose(1, 0, 2).reshape(P, 4 * NE)),
            'eoh': eoh_m, 'own_sel': osel,
            'qw': pk['qw'], 'qbt': pk['qbt'], 'kaw': pk['kaw'], 'kbw': pk['kbw'],
            'vaw': pk['vaw'], 'vbw': pk['vbw'], 'pjw': pk['pjw'], 'pjb': pk['pjb'],
            'ln1c': pk['ln1c'], 'ln2m': pk['ln2m'], 'rww': pk['rww'], 'rwb': pk['rwb'],
            'nww': pk['nww'], 'nwb': pk['nwb'],
            'w1t': w1tt, 'w2t': w2tt,
        }
        in_maps.append(m)
    return in_maps


def kernel(**inputs):
    if 'nc' not in _CACHE:
        _CACHE['nc'] = _build()
    nc = _CACHE['nc']
    in_maps = _prep_inputs(inputs)
    res = run_bass_kernel_spmd(nc, in_maps, core_ids=list(range(NCORE)))
    globals()['_LAST_RESULT'] = res
    out = np.concatenate([res.results[c]['out_x'] for c in range(NCORE)], 0)
    aux = np.float32(res.results[0]['out_aux'][0, 0])
    return out.reshape(B, T, D).astype(np.float32), aux


# revision 36
# speedup vs baseline: 1.0505x; 1.0505x over previous
"""Bass/Trainium2 kernel for nn_Block_57140244906730 (MLA attention + noisy top-2
router + 8 capacity-limited SwiGLU experts), distributed over 8 NeuronCores.

Sharding: token-parallel attention (512 tokens/core, kv computed per batch
block), expert-parallel MoE (1 expert/core) with AllGather dispatch/combine.
Capacity cumsum on device via tensor_tensor_scan + triangular matmuls;
dispatch/combine row movement via indirect DMA.
"""
import os
import sys

for _p in ('/root/.axon_site/_ro/trn_rl_repo', '/opt/trn_rl_repo'):
    if os.path.isdir(_p) and _p not in sys.path:
        sys.path.insert(0, _p)

import numpy as np
import ml_dtypes

import concourse.bacc as bacc
import concourse.bass as bass
import concourse.mybir as mybir
import concourse.tile as tile
from concourse.bass_utils import run_bass_kernel_spmd

f32 = mybir.dt.float32
bf16 = mybir.dt.bfloat16
i16 = mybir.dt.int16
i32 = mybir.dt.int32
AF = mybir.ActivationFunctionType
OP = mybir.AluOpType
BF = ml_dtypes.bfloat16

B, T, D = 4, 1024, 1024
NH, HD, LAT = 16, 64, 32
NE, TOPK = 8, 2
HID = 2730
HIDP = 2816           # padded hidden (22 * 128)
HC = HIDP // 128      # 22
N = B * T             # 4096
NCORE = 8
TOK = N // NCORE      # 512
TB = 1024             # batch-block (kv scope)
CAP = N * TOPK // NE  # 1024
DC = D // 128         # 8
P = 128
BIGNEG = 1e30

_CACHE = {}


def _rope(nc, pool, out_bf, in_f32, cos2_ap, sin2_ap):
    """RoPE with rows pre-permuted to [even(32); odd(32)].

    cos2 = [cos; cos], sin2 = [sin; -sin] (both [64, w]).
    tc = in*cos2; tsw = swap-halves(in*sin2)  (DMA partition swap);
    out = tc + tsw:
      out[0:32]  = e*cos + (o * -sin) = e*cos - o*sin
      out[32:64] = o*cos + (e *  sin) = o*cos + e*sin
    """
    tt = nc.vector.tensor_tensor
    w = in_f32.shape[-1]
    tc_ = pool.tile([HD, TB], f32, tag="rope_c")
    ts_ = pool.tile([HD, TB], f32, tag="rope_s")
    tw_ = pool.tile([HD, TB], f32, tag="rope_w")
    tt(tc_[:, :w], in_f32, cos2_ap, op=OP.mult)
    tt(ts_[:, :w], in_f32, sin2_ap, op=OP.mult)
    nc.sync.dma_start(tw_[0:32, :w], ts_[32:64, :w])
    nc.sync.dma_start(tw_[32:64, :w], ts_[0:32, :w])
    tt(out_bf[:, :w], tc_[:, :w], tw_[:, :w], op=OP.add)


def _build():
    nc = bacc.Bacc(None, target_bir_lowering=False, num_devices=NCORE)
    pr = lambda name, shape, dt: nc.declare_dram_parameter(name, list(shape), dt, isOutput=False)

    xq = pr("xq", [P, 4 * D], f32)
    xtb = pr("xtb", [P, DC * TB], f32)
    coskv = pr("coskv", [HD, TB], f32)   # [cos; cos]
    sinkv = pr("sinkv", [HD, TB], f32)   # [sin; -sin]
    epst = pr("epst", [P, 4 * NE], f32)
    eoh = pr("eoh", [P, NE], f32)
    own_sel = pr("own_sel", [P, 1], f32)
    qw = pr("qw", [P, NH * DC * HD], bf16)
    qbt = pr("qbt", [HD, NH], f32)
    kaw = pr("kaw", [P, NH * DC * LAT], bf16)
    kbw = pr("kbw", [LAT, NH * HD], bf16)
    vaw = pr("vaw", [P, NH * DC * LAT], bf16)
    vbw = pr("vbw", [LAT, NH * HD], bf16)
    pjw = pr("pjw", [P, DC * (NH // 2) * P], bf16)   # [p,(m,hp,c)]
    pjb = pr("pjb", [P, DC], f32)
    ln1c = pr("ln1c", [P, DC], f32)
    ln2m = pr("ln2m", [P, D], f32)
    rww = pr("rww", [P, DC * NE], f32)
    rwb = pr("rwb", [1, NE], f32)
    nww = pr("nww", [P, DC * NE], f32)
    nwb = pr("nwb", [1, NE], f32)
    w1t = pr("w1t", [HC, P, 2 * DC * P], bf16)
    w2t = pr("w2t", [P, HC * DC * P], bf16)

    out_x = nc.declare_dram_parameter("out_x", [TOK, D], f32, isOutput=True)
    out_aux = nc.declare_dram_parameter("out_aux", [1, 1], f32, isOutput=True)

    il = nc.inline_tensor
    su128 = il(np.triu(np.ones((P, P), np.float32), 1), name="su128")   # p'<p
    ui128 = il(np.triu(np.ones((P, P), np.float32), 0), name="ui128")   # p'<=p
    id128f = il(np.eye(P, dtype=np.float32), name="id128f")
    id128b = il(np.eye(P, dtype=BF), name="id128b")
    onc = il(np.ones((P, 1), np.float32), name="onc")
    onr = il(np.ones((1, P), np.float32), name="onr")
    onr64 = il(np.ones((1, 64), np.float32), name="onr64")
    er8c = il(np.tile(np.arange(NE, dtype=np.float32), (P, 1)), name="er8c")
    erm9c = il(np.tile(np.arange(NE, dtype=np.float32) - 9.0, (P, 1)), name="erm9c")
    ecm1c = il(np.tile(np.arange(NE, dtype=np.float32) * CAP - 1.0, (P, 1)), name="ecm1c")
    rep16 = il((np.arange(P)[None, :] % 16 == np.arange(16)[:, None]).astype(np.float32),
               name="rep16")

    agx_in = nc.dram_tensor("agx_in", [TOK, D], bf16)
    agx_out = nc.dram_tensor("agx_out", [N, D], bf16, addr_space="Shared")
    agp_in = nc.dram_tensor("agp_in", [TOK, NE], f32)
    agp_out = nc.dram_tensor("agp_out", [N, NE], f32, addr_space="Shared")
    ago_in = nc.dram_tensor("ago_in", [CAP, D], bf16)
    ago_out = nc.dram_tensor("ago_out", [NE * CAP, D], bf16, addr_space="Shared")
    # 32 independent slot->token scatter targets (one per scan column) so the
    # indirect scatters run in parallel; summed on-chip (disjoint writes + zeros)
    tokbufs = [nc.dram_tensor(f"tokb{j}", [CAP + 16], i16) for j in range(32)]

    RG = [list(range(NCORE))]
    mm = nc.tensor.matmul
    tt = nc.vector.tensor_tensor
    ts = nc.vector.tensor_scalar
    stt = nc.vector.scalar_tensor_tensor
    cp = nc.vector.tensor_copy
    act = nc.scalar.activation
    red = nc.vector.tensor_reduce
    X = mybir.AxisListType.X

    with tile.TileContext(nc) as tc:
        import contextlib
        with contextlib.ExitStack() as es:
            kc = es.enter_context(tc.tile_pool(name="consts", bufs=1))
            wt = es.enter_context(tc.tile_pool(name="wts", bufs=1))
            bigp = es.enter_context(tc.tile_pool(name="big", bufs=1))

            def load(pool, name, shape, dt, view):
                t = pool.tile(shape, dt, tag=name)
                nc.sync.dma_start(t[:], view)
                return t

            su_sb = load(kc, "su", [P, P], f32, su128[:])
            ui_sb = load(kc, "ui", [P, P], f32, ui128[:])
            idf_sb = load(kc, "idf", [P, P], f32, id128f[:])
            idb_sb = load(kc, "idb", [P, P], bf16, id128b[:])
            oc_sb = load(kc, "oc", [P, 1], f32, onc[:])
            or_sb = load(kc, "or", [1, P], f32, onr[:])
            # ones row placed at partition 64 (matmul lhsT/rhs base must match;
            # rhs is the softmax-denominator row living at partition 64)
            or64_sb = kc.tile([HD + 1, 64], f32, tag="or64")
            nc.sync.dma_start(or64_sb[HD:HD + 1, :], onr64[:])
            er8_sb = load(kc, "er8", [P, NE], f32, er8c[:])
            erm9_sb = load(kc, "erm9", [P, NE], f32, erm9c[:])
            ecm1_sb = load(kc, "ecm1", [P, NE], f32, ecm1c[:])
            eoh_sb = load(kc, "eoh", [P, NE], f32, eoh[:])
            osel_sb = load(kc, "osel", [P, 1], f32, own_sel[:])
            cos_sb = load(kc, "cos", [HD, TB], f32, coskv[:])
            sin_sb = load(kc, "sin", [HD, TB], f32, sinkv[:])
            ln1_sb = load(kc, "ln1", [P, DC], f32, ln1c[:])
            qbt_sb = load(kc, "qbt", [HD, NH], f32, qbt[:])
            pjb_sb = load(kc, "pjb", [P, DC], f32, pjb[:])
            rwb_sb = load(kc, "rwb", [1, NE], f32, rwb[:])
            nwb_sb = load(kc, "nwb", [1, NE], f32, nwb[:])
            eps_sb = load(kc, "eps", [P, 4, NE], f32, epst[:].rearrange("p (j e) -> p j e", j=4))
            kb_sb = load(kc, "kb", [LAT, NH, HD], bf16, kbw[:].rearrange("l (h e) -> l h e", h=NH))
            vb_sb = load(kc, "vb", [LAT, NH, HD], bf16, vbw[:].rearrange("l (h e) -> l h e", h=NH))

            # zero the 32 scatter buffers up front (no deps -> overlaps attention)
            ztok0 = kc.tile([1, CAP + 16], i16, tag="ztok0")
            nc.vector.memset(ztok0[:], 0)
            for _j in range(32):
                nc.sync.dma_start(tokbufs[_j][None, :], ztok0[:])

            ln2_sb = load(wt, "ln2", [P, D], f32, ln2m[:])
            rw_sb = load(wt, "rw", [P, DC, NE], f32, rww[:].rearrange("p (d e) -> p d e", d=DC))
            nw_sb = load(wt, "nw", [P, DC, NE], f32, nww[:].rearrange("p (d e) -> p d e", d=DC))

            x2r = bigp.tile([P, 4, D], f32, tag="x2r")
            probs = bigp.tile([P, 4, NE], f32, tag="probs")
            selt = bigp.tile([P, 4, NE], f32, tag="selt")
            idx_lo32 = bigp.tile([P, 4], i32, tag="idxlo")
            idx_hi32 = bigp.tile([P, 4], i32, tag="idxhi")
            gate_lo = bigp.tile([P, 4], f32, tag="glo")
            gate_hi = bigp.tile([P, 4], f32, tag="ghi")

            qw_v = qw[:].rearrange("p (h d e) -> p h d e", h=NH, d=DC)
            ka_v = kaw[:].rearrange("p (h d e) -> p h d e", h=NH, d=DC)
            va_v = vaw[:].rearrange("p (h d e) -> p h d e", h=NH, d=DC)
            pj_v = pjw[:].rearrange("p (m h c) -> p m h c", m=DC, h=NH // 2)
            xtb_v = xtb[:].rearrange("p (d t) -> p d t", d=DC)

            # ================= attention =================
            with tc.tile_pool(name="attp", bufs=1) as aw, \
                 tc.tile_pool(name="at1", bufs=1) as at1, \
                 tc.tile_pool(name="at2", bufs=2) as at2:
                xnT = aw.tile([P, DC, TB], bf16, tag="xnT")
                oTn = aw.tile([P, NH // 2, 512], bf16, tag="oTn")

                # ---- rms1 (xtb streamed) ----
                with tc.tile_pool(name="ps_r1", bufs=1, space="PSUM") as psr1:
                    ssq_ps = psr1.tile([1, TB], f32, tag="ssq")
                    for dc in range(DC):
                        xt = at2.tile([P, TB], f32, tag="xt")
                        nc.sync.dma_start(xt[:], xtb_v[:, dc, :])
                        sq = at2.tile([P, TB], f32, tag="sq")
                        tt(sq[:], xt[:], xt[:], op=OP.mult)
                        for hf in range(2):
                            mm(ssq_ps[:, 512 * hf:512 * (hf + 1)], lhsT=oc_sb[:],
                               rhs=sq[:, 512 * hf:512 * (hf + 1)],
                               start=(dc == 0), stop=(dc == DC - 1))
                    sq0 = at1.tile([1, TB], f32, tag="sq0")
                    ts(sq0[:], ssq_ps[:], 1.0 / D, 1e-5, op0=OP.mult, op1=OP.add)
                    act(sq0[:], sq0[:], AF.Sqrt)
                    rstd1 = at1.tile([1, TB], f32, tag="rstd1")
                    nc.vector.reciprocal(rstd1[:], sq0[:])
                    rb_ps = psr1.tile([P, TB], f32, tag="rb")
                    for hf in range(2):
                        mm(rb_ps[:, 512 * hf:512 * (hf + 1)], lhsT=or_sb[:],
                           rhs=rstd1[:, 512 * hf:512 * (hf + 1)], start=True, stop=True)
                    rstdb = at1.tile([P, TB], f32, tag="rstdb")
                    cp(rstdb[:], rb_ps[:])
                    for dc in range(DC):
                        xt = at2.tile([P, TB], f32, tag="xt")
                        nc.sync.dma_start(xt[:], xtb_v[:, dc, :])
                        stt(xnT[:, dc, :], xt[:], ln1_sb[:, dc:dc + 1], rstdb[:],
                            op0=OP.mult, op1=OP.mult)

                # ---- heads ----
                with tc.tile_pool(name="ps_h", bufs=1, space="PSUM") as psh, \
                     tc.tile_pool(name="ps_h2", bufs=2, space="PSUM") as psh2:
                    for h in range(NH):
                        qw_h = at2.tile([P, DC, HD], bf16, tag="qw_h")
                        nc.sync.dma_start(qw_h[:], qw_v[:, h, :, :])
                        ka_h = at2.tile([P, DC, LAT], bf16, tag="ka_h")
                        nc.sync.dma_start(ka_h[:], ka_v[:, h, :, :])
                        va_h = at2.tile([P, DC, LAT], bf16, tag="va_h")
                        nc.sync.dma_start(va_h[:], va_v[:, h, :, :])

                        q_ps = psh.tile([HD, 512], f32, tag="q_ps")
                        for dc in range(DC):
                            mm(q_ps[:], lhsT=qw_h[:, dc, :], rhs=xnT[:, dc, 0:512],
                               start=(dc == 0), stop=(dc == DC - 1))
                        q_sb = at1.tile([HD, TB], f32, tag="qk_pre")
                        act(q_sb[:, 0:512], q_ps[:], AF.Identity, bias=qbt_sb[:, h:h + 1])
                        qr = at2.tile([HD, 512], bf16, tag="qr")
                        _rope(nc, at1, qr, q_sb[:, 0:512], cos_sb[:, 0:512], sin_sb[:, 0:512])

                        latk = at2.tile([LAT, TB], bf16, tag="latk")
                        for hf in range(2):
                            lat_ps = psh.tile([LAT, 512], f32, tag="lat_ps")
                            for dc in range(DC):
                                mm(lat_ps[:], lhsT=ka_h[:, dc, :],
                                   rhs=xnT[:, dc, 512 * hf:512 * (hf + 1)],
                                   start=(dc == 0), stop=(dc == DC - 1))
                            cp(latk[:, 512 * hf:512 * (hf + 1)], lat_ps[:])
                        k_sb = at1.tile([HD, TB], f32, tag="k_pre")
                        for hf in range(2):
                            kp_ps = psh.tile([HD, 512], f32, tag="kp_ps")
                            mm(kp_ps[:], lhsT=kb_sb[:, h, :],
                               rhs=latk[:, 512 * hf:512 * (hf + 1)], start=True, stop=True)
                            cp(k_sb[:, 512 * hf:512 * (hf + 1)], kp_ps[:])
                        kr = at2.tile([HD, TB], bf16, tag="kr")
                        _rope(nc, at1, kr, k_sb[:], cos_sb[:], sin_sb[:])

                        latv = at2.tile([LAT, TB], bf16, tag="latv")
                        for hf in range(2):
                            lat_ps = psh.tile([LAT, 512], f32, tag="lat_ps")
                            for dc in range(DC):
                                mm(lat_ps[:], lhsT=va_h[:, dc, :],
                                   rhs=xnT[:, dc, 512 * hf:512 * (hf + 1)],
                                   start=(dc == 0), stop=(dc == DC - 1))
                            cp(latv[:, 512 * hf:512 * (hf + 1)], lat_ps[:])
                        # vaug cols 0:64 = v, col 64 = ones (denominator row at
                        # partition 64 of oT_ps)
                        vaug = at2.tile([P, 8, HD + 1], bf16, tag="vaug")
                        for kcc in range(8):
                            v_ps = psh.tile([P, 512], f32, tag="aux_ps")
                            mm(v_ps[:, 0:HD], lhsT=latv[:, 128 * kcc:128 * (kcc + 1)],
                               rhs=vb_sb[:, h, :], start=True, stop=True)
                            cp(vaug[:, kcc, 0:HD], v_ps[:, 0:HD])
                        nc.vector.memset(vaug[:, :, HD:HD + 1], 1.0)

                        p_sb = at2.tile([P, 8, 512], bf16, tag="p_sb")
                        for kcc in range(8):
                            s_ps = psh2.tile([P, 512], f32, tag="s_ps")
                            mm(s_ps[:], lhsT=kr[:, 128 * kcc:128 * (kcc + 1)], rhs=qr[:],
                               start=True, stop=True)
                            act(p_sb[:, kcc, :], s_ps[:], AF.Exp, scale=0.125)
                        oT_ps = psh.tile([HD + 1, 512], f32, tag="oT_ps")
                        for kcc in range(8):
                            mm(oT_ps[:], lhsT=vaug[:, kcc, :], rhs=p_sb[:, kcc, :],
                               start=(kcc == 0), stop=(kcc == 7))
                        rs64 = at1.tile([HD + 1, 512], f32, tag="rs64")
                        nc.vector.reciprocal(rs64[HD:HD + 1, :], oT_ps[HD:HD + 1, :])
                        rb2_ps = psh.tile([P, 512], f32, tag="aux_ps")
                        mm(rb2_ps[0:HD, :], lhsT=or64_sb[HD:HD + 1, :], rhs=rs64[HD:HD + 1, :],
                           start=True, stop=True)
                        rb2 = at1.tile([HD, 512], f32, tag="rb2s")
                        cp(rb2[:], rb2_ps[0:HD, :])
                        oTn_h = at2.tile([HD, 512], bf16, tag="oTn_h")
                        tt(oTn_h[:], oT_ps[0:HD, :], rb2[:], op=OP.mult)
                        r0 = (h % 2) * HD
                        nc.sync.dma_start(oTn[r0:r0 + HD, h // 2, :], oTn_h[:])

                # ---- proj + residual (rows) ----
                with tc.tile_pool(name="ps_pj", bufs=1, space="PSUM") as pspj, \
                     tc.tile_pool(name="ps_tr", bufs=2, space="PSUM") as pstr:
                    # proj computed per m (all tokens), transposed per t4
                    pjT_all = aw.tile([P, DC, 512], bf16, tag="pjT_all")
                    for m in range(DC):
                        pj_sb_m = at2.tile([P, NH // 2, P], bf16, tag="pj_w")
                        nc.sync.dma_start(pj_sb_m[:], pj_v[:, m, :, :])
                        pj_ps = pspj.tile([P, 512], f32, tag="pj_ps")
                        for hp in range(NH // 2):
                            mm(pj_ps[:], lhsT=pj_sb_m[:, hp, :], rhs=oTn[:, hp, :],
                               start=(hp == 0), stop=(hp == NH // 2 - 1))
                        act(pjT_all[:, m, :], pj_ps[:], AF.Identity, bias=pjb_sb[:, m:m + 1])
                    for t4 in range(4):
                        xqt = at2.tile([P, D], f32, tag="xqt")
                        nc.sync.dma_start(xqt[:], xq[:].rearrange("p (j d) -> p j d", j=4)[:, t4, :])
                        for m in range(DC):
                            tr_ps = pstr.tile([P, P], bf16, tag="tr")
                            nc.tensor.transpose(tr_ps[:], pjT_all[:, m, 128 * t4:128 * (t4 + 1)], idb_sb[:])
                            tt(x2r[:, t4, 128 * m:128 * (m + 1)], tr_ps[:],
                               xqt[:, 128 * m:128 * (m + 1)], op=OP.add)

            # ================= rms2 / xn2 / AG(x) / router =================
            with tc.tile_pool(name="n2", bufs=2) as n2p, \
                 tc.tile_pool(name="n2s", bufs=1) as n2s, \
                 tc.tile_pool(name="xn2Tp", bufs=1) as xtp, \
                 tc.tile_pool(name="ps_n2", bufs=2, space="PSUM") as psn:
                xn2T = xtp.tile([P, DC, 512], f32, tag="xn2T")
                for j2 in range(4):
                    sq = n2p.tile([P, D], f32, tag="sq2")
                    tt(sq[:], x2r[:, j2, :], x2r[:, j2, :], op=OP.mult)
                    ssq = n2p.tile([P, 1], f32, tag="ssq2")
                    red(ssq[:], sq[:], axis=X, op=OP.add)
                    rstd2 = n2p.tile([P, 1], f32, tag="rstd2")
                    ts(rstd2[:], ssq[:], 1.0 / D, 1e-5, op0=OP.mult, op1=OP.add)
                    act(rstd2[:], rstd2[:], AF.Sqrt)
                    nc.vector.reciprocal(rstd2[:], rstd2[:])
                    xn2 = n2p.tile([P, D], f32, tag="xn2")
                    stt(xn2[:], x2r[:, j2, :], rstd2[:], ln2_sb[:], op0=OP.mult, op1=OP.mult)
                    xn2b = n2p.tile([P, D], bf16, tag="xn2b")
                    cp(xn2b[:], xn2[:])
                    nc.sync.dma_start(agx_in[128 * j2:128 * (j2 + 1), :], xn2b[:])
                    for m in range(DC):
                        tr_ps = psn.tile([P, P], f32, tag="trn")
                        nc.tensor.transpose(tr_ps[:], xn2[:, 128 * m:128 * (m + 1)], idf_sb[:])
                        cp(xn2T[:, m, 128 * j2:128 * (j2 + 1)], tr_ps[:])
                nc.gpsimd.collective_compute(
                    "AllGather", OP.bypass, replica_groups=RG,
                    ins=[agx_in[:]], outs=[agx_out[:]])

                for j2 in range(4):
                    lg_ps = psn.tile([P, NE], f32, tag="lg")
                    npp_ps = psn.tile([P, NE], f32, tag="npp")
                    for dc in range(DC):
                        mm(lg_ps[:], lhsT=xn2T[:, dc, 128 * j2:128 * (j2 + 1)],
                           rhs=rw_sb[:, dc, :], start=(dc == 0), stop=False)
                        mm(npp_ps[:], lhsT=xn2T[:, dc, 128 * j2:128 * (j2 + 1)],
                           rhs=nw_sb[:, dc, :], start=(dc == 0), stop=False)
                    mm(lg_ps[:], lhsT=or_sb[:], rhs=rwb_sb[:], start=False, stop=True)
                    mm(npp_ps[:], lhsT=or_sb[:], rhs=nwb_sb[:], start=False, stop=True)
                    # softplus via Newton on e^y = 1 + e^x (no Softplus table on
                    # this toolchain): y_{n+1} = y_n - 1 + (1+e^x) e^{-y_n}
                    a_t = n2p.tile([P, NE], f32, tag="a_t")
                    act(a_t[:], npp_ps[:], AF.Exp)
                    ts(a_t[:], a_t[:], 1.0, None, op0=OP.add)
                    sp = n2p.tile([P, NE], f32, tag="sp")
                    act(sp[:], npp_ps[:], AF.Relu)
                    for _it in range(5):
                        t_ = n2p.tile([P, NE], f32, tag="t_nwt")
                        act(t_[:], sp[:], AF.Exp, scale=-1.0)
                        tt(t_[:], t_[:], a_t[:], op=OP.mult)
                        stt(sp[:], t_[:], -1.0, sp[:], op0=OP.add, op1=OP.add)
                    noisy = n2p.tile([P, NE], f32, tag="noisy")
                    tt(noisy[:], eps_sb[:, j2, :], sp[:], op=OP.mult)
                    tt(noisy[:], noisy[:], lg_ps[:], op=OP.add)
                    m1 = n2p.tile([P, 1], f32, tag="m1")
                    red(m1[:], noisy[:], axis=X, op=OP.max)
                    eq = n2p.tile([P, NE], f32, tag="eq")
                    ts(eq[:], noisy[:], m1[:], None, op0=OP.is_equal)
                    msk = n2p.tile([P, NE], f32, tag="msk")
                    stt(msk[:], eq[:], -BIGNEG, noisy[:], op0=OP.mult, op1=OP.add)
                    m2 = n2p.tile([P, 1], f32, tag="m2")
                    red(m2[:], msk[:], axis=X, op=OP.max)
                    ts(selt[:, j2, :], noisy[:], m2[:], None, op0=OP.is_ge)
                    m1n = n2p.tile([P, 1], f32, tag="m1n")
                    ts(m1n[:], m1[:], -1.0, None, op0=OP.mult)
                    pe = n2p.tile([P, NE], f32, tag="pe")
                    act(pe[:], noisy[:], AF.Exp, bias=m1n[:])
                    tt(pe[:], pe[:], selt[:, j2, :], op=OP.mult)
                    z = n2p.tile([P, 1], f32, tag="z")
                    red(z[:], pe[:], axis=X, op=OP.add)
                    rz = n2p.tile([P, 1], f32, tag="rz")
                    nc.vector.reciprocal(rz[:], z[:])
                    ts(probs[:, j2, :], pe[:], rz[:], None, op0=OP.mult)
                    pjo = n2p.tile([P, NE], f32, tag="pjo")
                    cp(pjo[:], probs[:, j2, :])
                    nc.sync.dma_start(agp_in[128 * j2:128 * (j2 + 1), :], pjo[:])
                nc.gpsimd.collective_compute(
                    "AllGather", OP.bypass, replica_groups=RG,
                    ins=[agp_in[:]], outs=[agp_out[:]])

            # ================= global routing / capacity / aux =================
            with tc.tile_pool(name="rt", bufs=1) as rt, \
                 tc.tile_pool(name="rt2", bufs=4) as rt2, \
                 tc.tile_pool(name="ps_rt", bufs=1, space="PSUM") as psr, \
                 tc.tile_pool(name="ps_rt2", bufs=2, space="PSUM") as psr2:
                pg = rt.tile([P, 32, NE], f32, tag="pg")
                nc.sync.dma_start(pg[:], agp_out[:].rearrange("(p j) e -> p j e", p=P))
                peo = rt.tile([P, NE, 32], f32, tag="peo")
                cp(peo[:], pg[:].transpose([0, 2, 1]))
                meo = rt.tile([P, NE, 32], f32, tag="meo")
                ts(meo[:], peo[:], 0.0, None, op0=OP.is_gt)
                tots = rt.tile([P, NE], f32, tag="tots")
                red(tots[:], meo[:], axis=X, op=OP.add)
                gpoff_ps = psr.tile([P, NE], f32, tag="gpoff")
                mm(gpoff_ps[:], lhsT=su_sb[:], rhs=tots[:], start=True, stop=True)
                gpoff = rt.tile([P, NE], f32, tag="gpoffs")
                cp(gpoff[:], gpoff_ps[:])

                tmpe = rt.tile([P, 32, NE], f32, tag="tmpe")
                tt(tmpe[:], pg[:], eoh_sb[:].unsqueeze(1).broadcast_to([P, 32, NE]), op=OP.mult)
                gmine = rt.tile([P, 32], f32, tag="gmine")
                red(gmine[:], tmpe[:], axis=X, op=OP.add)
                m_mine = rt.tile([P, 32], f32, tag="mmine")
                ts(m_mine[:], gmine[:], 0.0, None, op0=OP.is_gt)
                scan = rt.tile([P, 32], f32, tag="scan")
                nc.vector.tensor_tensor_scan(scan[:], m_mine[:], m_mine[:], 0.0,
                                             op0=OP.add, op1=OP.bypass)
                gpm = rt.tile([P, NE], f32, tag="gpm")
                tt(gpm[:], gpoff[:], eoh_sb[:], op=OP.mult)
                gpm1 = rt.tile([P, 1], f32, tag="gpm1")
                red(gpm1[:], gpm[:], axis=X, op=OP.add)
                incl = rt.tile([P, 32], f32, tag="incl")
                ts(incl[:], scan[:], gpm1[:], None, op0=OP.add)
                slot = rt.tile([P, 32], f32, tag="slot")
                ts(slot[:], incl[:], -1.0, None, op0=OP.add)
                ts(slot[:], slot[:], float(CAP), None, op0=OP.min)
                ts(slot[:], slot[:], -float(CAP), None, op0=OP.add)
                tt(slot[:], slot[:], m_mine[:], op=OP.mult)
                ts(slot[:], slot[:], float(CAP), None, op0=OP.add)
                # --- tok list: 32 parallel per-column indirect scatters into
                # 32 zeroed buffers, later summed (disjoint nonzeros) ---
                slot32 = rt.tile([P, 32], i32, tag="slot32")
                cp(slot32[:], slot[:])
                iot = rt.tile([P, 32], i16, tag="iot")
                nc.gpsimd.iota(iot[:], pattern=[[1, 32]], base=0, channel_multiplier=32)
                for j in range(32):
                    nc.gpsimd.indirect_dma_start(
                        out=tokbufs[j][:, None],
                        out_offset=bass.IndirectOffsetOnAxis(ap=slot32[:, j:j + 1], axis=0),
                        in_=iot[:, j:j + 1], in_offset=None)
                # aux loss (identical on every core)
                jsum = rt.tile([P, NE], f32, tag="jsum")
                red(jsum[:], peo[:], axis=X, op=OP.add)
                cs_ps = psr.tile([1, NE], f32, tag="cs")
                mm(cs_ps[:], lhsT=oc_sb[:], rhs=jsum[:], start=True, stop=True)
                auxv = rt.tile([1, NE], f32, tag="auxv")
                ts(auxv[:], cs_ps[:], 1.0 / N, -1.0 / NE, op0=OP.mult, op1=OP.add)
                act(auxv[:], auxv[:], AF.Square)
                aux1 = rt.tile([1, 1], f32, tag="aux1")
                red(aux1[:], auxv[:], axis=X, op=OP.add)
                nc.sync.dma_start(out_aux[:], aux1[:])

                # combine-side indices (own tokens, tile-major)
                mown = rt.tile([P, 4, NE], f32, tag="mown")
                ts(mown[:], probs[:], 0.0, None, op0=OP.is_gt)
                inclo = rt.tile([P, 4, NE], f32, tag="inclo")
                coffacc_ps = psr.tile([1, NE], f32, tag="coffacc")
                mm(coffacc_ps[:], lhsT=osel_sb[:], rhs=gpoff[:], start=True, stop=False)
                for j2 in range(4):
                    coff = rt2.tile([1, NE], f32, tag="coff")
                    cp(coff[:], coffacc_ps[:])
                    mj = rt2.tile([P, NE], f32, tag="mj")
                    cp(mj[:], mown[:, j2, :])
                    csc_ps = psr2.tile([P, NE], f32, tag="csc")
                    mm(csc_ps[:], lhsT=ui_sb[:], rhs=mj[:], start=True, stop=False)
                    mm(csc_ps[:], lhsT=or_sb[:], rhs=coff[:], start=False, stop=True)
                    cp(inclo[:, j2, :], csc_ps[:])
                    if j2 < 3:
                        mm(coffacc_ps[:], lhsT=oc_sb[:], rhs=mj[:], start=False,
                           stop=(j2 == 2))
                keep = rt.tile([P, 4, NE], f32, tag="keep")
                ts(keep[:], inclo[:], float(CAP), None, op0=OP.is_le)
                tt(keep[:], keep[:], mown[:], op=OP.mult)
                gate0 = rt.tile([P, 4, NE], f32, tag="gate0")
                tt(gate0[:], probs[:], keep[:], op=OP.mult)
                idxc = rt.tile([P, 4, NE], f32, tag="idxc")
                ts(idxc[:], inclo[:], float(CAP), None, op0=OP.min)
                tt(idxc[:], idxc[:], ecm1_sb[:].unsqueeze(1).broadcast_to([P, 4, NE]), op=OP.add)
                erb = er8_sb[:].unsqueeze(1).broadcast_to([P, 4, NE])
                u = rt.tile([P, 4, NE], f32, tag="u")
                tt(u[:], erm9_sb[:].unsqueeze(1).broadcast_to([P, 4, NE]), selt[:], op=OP.mult)
                ts(u[:], u[:], 9.0, None, op0=OP.add)
                elo = rt.tile([P, 4], f32, tag="elo")
                red(elo[:], u[:], axis=X, op=OP.min)
                wv = rt.tile([P, 4, NE], f32, tag="wv")
                tt(wv[:], erb, selt[:], op=OP.mult)
                ehi = rt.tile([P, 4], f32, tag="ehi")
                red(ehi[:], wv[:], axis=X, op=OP.max)
                for nm, ev, idxo, gto in (("lo", elo, idx_lo32, gate_lo),
                                          ("hi", ehi, idx_hi32, gate_hi)):
                    oh = rt.tile([P, 4, NE], f32, tag="oh" + nm)
                    tt(oh[:], erb, ev[:].unsqueeze(2).broadcast_to([P, 4, NE]), op=OP.is_equal)
                    tmp = rt.tile([P, 4, NE], f32, tag="tmq" + nm)
                    tt(tmp[:], gate0[:], oh[:], op=OP.mult)
                    red(gto[:], tmp[:], axis=X, op=OP.add)
                    tt(tmp[:], idxc[:], oh[:], op=OP.mult)
                    idxf = rt2.tile([P, 4], f32, tag="idxf")
                    red(idxf[:], tmp[:], axis=X, op=OP.add)
                    cp(idxo[:], idxf[:])

            # ================= expert =================
            with tc.tile_pool(name="ex", bufs=1) as ex, \
                 tc.tile_pool(name="w1p", bufs=2) as w1p, \
                 tc.tile_pool(name="ex2", bufs=2) as ex2, \
                 tc.tile_pool(name="oerp", bufs=2) as oerp, \
                 tc.tile_pool(name="ps_e", bufs=2, space="PSUM") as pse, \
                 tc.tile_pool(name="ps_e2", bufs=2, space="PSUM") as pse2:
                w2_sb = ex.tile([P, HC, DC, P], bf16, tag="w2")
                nc.sync.dma_start(w2_sb[:], w2t[:].rearrange("p (h m c) -> p h m c", h=HC, m=DC))
                # sum the 32 scatter buffers (each zero except its own slots)
                tk0 = ex.tile([P, 8], i16, tag="tk0")
                nc.sync.dma_start(tk0[:], tokbufs[0][0:CAP].rearrange("(p j) -> p j", p=P))
                tok16 = ex.tile([P, 8], i16, tag="tok16")
                tk1 = ex2.tile([P, 8], i16, tag="tk1")
                nc.sync.dma_start(tk1[:], tokbufs[1][0:CAP].rearrange("(p j) -> p j", p=P))
                tt(tok16[:], tk0[:], tk1[:], op=OP.add)
                for j in range(2, 32):
                    tkj = ex2.tile([P, 8], i16, tag="tk1")
                    nc.sync.dma_start(tkj[:], tokbufs[j][0:CAP].rearrange("(p j) -> p j", p=P))
                    tt(tok16[:], tok16[:], tkj[:], op=OP.add)
                tok32 = ex.tile([P, 8], i32, tag="tok32")
                cp(tok32[:], tok16[:])
                xeT = ex.tile([P, DC, CAP], bf16, tag="xeT")
                for j in range(8):
                    xe = ex2.tile([P, D], bf16, tag="xe")
                    nc.gpsimd.indirect_dma_start(
                        out=xe[:], out_offset=None, in_=agx_out[:],
                        in_offset=bass.IndirectOffsetOnAxis(ap=tok32[:, j:j + 1], axis=0))
                    for dc in range(DC):
                        tr_ps = pse2.tile([P, P], bf16, tag="trx")
                        nc.tensor.transpose(tr_ps[:], xe[:, 128 * dc:128 * (dc + 1)], idb_sb[:])
                        cp(xeT[:, dc, 128 * j:128 * (j + 1)], tr_ps[:])
                g_sb = ex.tile([P, HC, 2, 512], bf16, tag="g")
                for hc in range(HC):
                    w1_sb = w1p.tile([P, 2, DC, P], bf16, tag="w1")
                    nc.sync.dma_start(w1_sb[:], w1t[hc].rearrange("p (h d c) -> p h d c", h=2, d=DC))
                    for t5 in range(2):
                        h1_ps = pse.tile([P, 512], f32, tag="h1")
                        h2_ps = pse.tile([P, 512], f32, tag="h2")
                        for dc in range(DC):
                            mm(h1_ps[:], lhsT=w1_sb[:, 0, dc, :],
                               rhs=xeT[:, dc, 512 * t5:512 * (t5 + 1)],
                               start=(dc == 0), stop=(dc == DC - 1))
                            mm(h2_ps[:], lhsT=w1_sb[:, 1, dc, :],
                               rhs=xeT[:, dc, 512 * t5:512 * (t5 + 1)],
                               start=(dc == 0), stop=(dc == DC - 1))
                        sil = ex2.tile([P, 512], f32, tag="sil")
                        act(sil[:], h1_ps[:], AF.Silu)
                        tt(g_sb[:, hc, t5, :], sil[:], h2_ps[:], op=OP.mult)
                ago_v = ago_in[:].rearrange("(p j) d -> p j d", p=P)
                for t5 in range(2):
                    oers = [oerp.tile([P, D], bf16, tag=f"oer{q}", name=f"oer{q}_{t5}")
                            for q in range(4)]
                    for m in range(DC):
                        oe_ps = pse.tile([P, 512], f32, tag="oe")
                        for hc in range(HC):
                            mm(oe_ps[:], lhsT=w2_sb[:, hc, m, :], rhs=g_sb[:, hc, t5, :],
                               start=(hc == 0), stop=(hc == HC - 1))
                        oeT = ex2.tile([P, 512], bf16, tag="oeT")
                        cp(oeT[:], oe_ps[:])
                        for q in range(4):
                            tr_ps = pse2.tile([P, P], bf16, tag="trx")
                            nc.tensor.transpose(tr_ps[:], oeT[:, 128 * q:128 * (q + 1)], idb_sb[:])
                            cp(oers[q][:, 128 * m:128 * (m + 1)], tr_ps[:])
                    # slot s = 8*p + j' with j' = 4*t5 + q  (tok loaded as [p, j])
                    for q in range(4):
                        nc.sync.dma_start(ago_v[:, 4 * t5 + q, :], oers[q][:])
                nc.gpsimd.collective_compute(
                    "AllGather", OP.bypass, replica_groups=RG,
                    ins=[ago_in[:]], outs=[ago_out[:]])

            # ================= combine =================
            with tc.tile_pool(name="cb", bufs=2) as cb:
                for j2 in range(4):
                    glo = cb.tile([P, D], bf16, tag="glo_t")
                    nc.gpsimd.indirect_dma_start(
                        out=glo[:], out_offset=None, in_=ago_out[:],
                        in_offset=bass.IndirectOffsetOnAxis(ap=idx_lo32[:, j2:j2 + 1], axis=0))
                    ghi = cb.tile([P, D], bf16, tag="ghi_t")
                    nc.gpsimd.indirect_dma_start(
                        out=ghi[:], out_offset=None, in_=ago_out[:],
                        in_offset=bass.IndirectOffsetOnAxis(ap=idx_hi32[:, j2:j2 + 1], axis=0))
                    acc = cb.tile([P, D], f32, tag="acc")
                    stt(acc[:], glo[:], gate_lo[:, j2:j2 + 1], x2r[:, j2, :],
                        op0=OP.mult, op1=OP.add)
                    out_t = cb.tile([P, D], f32, tag="out_t")
                    stt(out_t[:], ghi[:], gate_hi[:, j2:j2 + 1], acc[:],
                        op0=OP.mult, op1=OP.add)
                    nc.sync.dma_start(out_x[128 * j2:128 * (j2 + 1), :], out_t[:])

    nc.compile()
    return nc


# ======================= host side =======================

def _rope_tables():
    inv = 1.0 / (10000.0 ** (np.arange(0, HD, 2, dtype=np.float32) / HD))
    ang = np.arange(T, dtype=np.float32)[:, None] * inv[None, :]
    return np.cos(ang).astype(np.float32), np.sin(ang).astype(np.float32)


def _perm_cols(a):
    idx = np.concatenate([np.arange(0, HD, 2), np.arange(1, HD, 2)])
    return a[..., idx]


def _prep_shared(inp):
    qwp = _perm_cols(np.asarray(inp['q_w'], np.float32))
    qbp = _perm_cols(np.asarray(inp['q_b'], np.float32))
    kbp = _perm_cols(np.asarray(inp['kb_w'], np.float32))
    cos, sin = _rope_tables()

    pk = {}
    pk['qw'] = np.ascontiguousarray(
        qwp.reshape(NH, DC, P, HD).transpose(2, 0, 1, 3).reshape(P, NH * DC * HD)).astype(BF)
    pk['qbt'] = np.ascontiguousarray(qbp.T).astype(np.float32)
    pk['kaw'] = np.ascontiguousarray(
        np.asarray(inp['ka_w'], np.float32).reshape(NH, DC, P, LAT).transpose(2, 0, 1, 3).reshape(P, NH * DC * LAT)).astype(BF)
    pk['vaw'] = np.ascontiguousarray(
        np.asarray(inp['va_w'], np.float32).reshape(NH, DC, P, LAT).transpose(2, 0, 1, 3).reshape(P, NH * DC * LAT)).astype(BF)
    pk['kbw'] = np.ascontiguousarray(kbp.transpose(1, 0, 2).reshape(LAT, NH * HD)).astype(BF)
    pk['vbw'] = np.ascontiguousarray(
        np.asarray(inp['vb_w'], np.float32).transpose(1, 0, 2).reshape(LAT, NH * HD)).astype(BF)
    # pjw[p, m, hp, c] = proj_w[128*hp + p, 128*m + c]
    pk['pjw'] = np.ascontiguousarray(
        np.asarray(inp['proj_w'], np.float32).reshape(NH // 2, P, DC, P)
        .transpose(1, 2, 0, 3).reshape(P, DC * (NH // 2) * P)).astype(BF)
    pk['pjb'] = np.ascontiguousarray(np.asarray(inp['proj_b'], np.float32).reshape(DC, P).T)
    pk['ln1c'] = np.ascontiguousarray(np.asarray(inp['ln1_w'], np.float32).reshape(DC, P).T)
    pk['ln2m'] = np.tile(np.asarray(inp['ln2_w'], np.float32)[None, :], (P, 1))
    pk['rww'] = np.ascontiguousarray(
        np.asarray(inp['route_w'], np.float32).reshape(DC, P, NE).transpose(1, 0, 2).reshape(P, DC * NE))
    pk['nww'] = np.ascontiguousarray(
        np.asarray(inp['noise_w'], np.float32).reshape(DC, P, NE).transpose(1, 0, 2).reshape(P, DC * NE))
    pk['rwb'] = np.asarray(inp['route_b'], np.float32).reshape(1, NE)
    pk['nwb'] = np.asarray(inp['noise_b'], np.float32).reshape(1, NE)
    pk['cos'] = cos
    pk['sin'] = sin
    return pk


def _prep_expert(inp, e):
    w1 = np.asarray(inp['swiglu_w'][e], np.float32)
    w2 = np.asarray(inp['down_w'][e], np.float32)
    h1 = np.zeros((D, HIDP), np.float32); h1[:, :HID] = w1[:, :HID]
    h2 = np.zeros((D, HIDP), np.float32); h2[:, :HID] = w1[:, HID:]
    w2p = np.zeros((HIDP, D), np.float32); w2p[:HID] = w2
    h1r = h1.reshape(DC, P, HC, P)
    h2r = h2.reshape(DC, P, HC, P)
    w1tt = np.stack([h1r, h2r], axis=0)          # [half, dc, p, hc, c]
    w1tt = w1tt.transpose(3, 2, 0, 1, 4)         # [hc, p, half, dc, c]
    w1tt = np.ascontiguousarray(w1tt.reshape(HC, P, 2 * DC * P)).astype(BF)
    w2r = w2p.reshape(HC, P, DC, P).transpose(1, 0, 2, 3)
    w2tt = np.ascontiguousarray(w2r.reshape(P, HC * DC * P)).astype(BF)
    return w1tt, w2tt


def _prep_inputs(inp):
    x = np.asarray(inp['x'], np.float32)
    eps = np.asarray(inp['noise_eps'], np.float32).reshape(N, NE)
    pk = _prep_shared(inp)
    cos, sin = pk['cos'], pk['sin']
    in_maps = []
    for c in range(NCORE):
        b, half = c // 2, c % 2
        s_own = slice(512 * half, 512 * (half + 1))
        s_oth = slice(512 * (1 - half), 512 * (2 - half))
        xb = x[b]
        xq_own = xb[s_own]
        perm_cos = np.concatenate([cos[s_own], cos[s_oth]], 0)
        perm_sin = np.concatenate([sin[s_own], sin[s_oth]], 0)
        xtb_own = np.concatenate([xb[s_own], xb[s_oth]], 0)
        w1tt, w2tt = _prep_expert(inp, c)
        eoh_m = np.zeros((P, NE), np.float32); eoh_m[:, c] = 1.0
        osel = np.zeros((P, 1), np.float32); osel[16 * c, 0] = 1.0
        m = {
            'xq': np.ascontiguousarray(xq_own.reshape(4, P, D).transpose(1, 0, 2).reshape(P, 4 * D)),
            'xtb': np.ascontiguousarray(xtb_own.T.reshape(DC, P, TB).transpose(1, 0, 2).reshape(P, DC * TB)),
            'coskv': np.ascontiguousarray(np.concatenate([perm_cos.T, perm_cos.T], 0)),
            'sinkv': np.ascontiguousarray(np.concatenate([perm_sin.T, -perm_sin.T], 0)),
            'epst': np.ascontiguousarray(
                eps[512 * c:512 * (c + 1)].reshape(4, P, NE).transpose(1, 0, 2).reshape(P, 4 * NE)),
            'eoh': eoh_m, 'own_sel': osel,
            'qw': pk['qw'], 'qbt': pk['qbt'], 'kaw': pk['kaw'], 'kbw': pk['kbw'],
            'vaw': pk['vaw'], 'vbw': pk['vbw'], 'pjw': pk['pjw'], 'pjb': pk['pjb'],
            'ln1c': pk['ln1c'], 'ln2m': pk['ln2m'], 'rww': pk['rww'], 'rwb': pk['rwb'],
            'nww': pk['nww'], 'nwb': pk['nwb'],
            'w1t': w1tt, 'w2t': w2tt,
        }
        in_maps.append(m)
    return in_maps


def _ensure_ntff_hook():
    """BASS_TRACE=1 needs antenv.axon_hooks, absent on this image; inject it."""
    import types
    try:
        import antenv.axon_hooks  # noqa: F401
        return
    except ImportError:
        pass
    try:
        from trn_agent_boot.trn_boot import _ntff_profile_via_ctypes
        import antenv
        mod = types.ModuleType('antenv.axon_hooks')
        hook = _ntff_profile_via_ctypes('/opt/axon/libaxon_pjrt.so')
        mod.get_axon_ntff_profile_hook = lambda: hook
        mod.set_axon_ntff_profile_hook = lambda h: None
        sys.modules['antenv.axon_hooks'] = mod
        antenv.axon_hooks = mod
    except Exception:
        os.environ['BASS_NEVER_TRACE'] = '1'


def kernel(**inputs):
    _ensure_ntff_hook()
    if 'nc' not in _CACHE:
        _CACHE['nc'] = _build()
    nc = _CACHE['nc']
    in_maps = _prep_inputs(inputs)
    res = run_bass_kernel_spmd(nc, in_maps, core_ids=list(range(NCORE)))
    globals()['_LAST_RESULT'] = res
    out = np.concatenate([res.results[c]['out_x'] for c in range(NCORE)], 0)
    aux = np.float32(res.results[0]['out_aux'][0, 0])
    return out.reshape(B, T, D).astype(np.float32), aux
